# revision 4
# baseline (speedup 1.0000x reference)
"""DeepSeekMoE kernel for 8 TRN2 NeuronCores.

Sharding: load-balanced expert-parallel. Each routed expert's FFN is split
in half along the hidden (H) axis across two cores, and the 4 heaviest
experts (by routed-token count) are paired with the 4 lightest, so every
core carries one heavy half-expert (slot A) and one light half-expert
(slot B) — per-core matmul rows become nearly uniform instead of every
core paying the max expert's padding. Each core also owns a 1/8 H-shard
of the shared expert (tensor-parallel).

The tiny gate (sigmoid + top-2 over E=8) runs on host; tokens are gathered
per expert, padded to ntc*tsz (equal token-chunks <=512, SPMD: one program
for all 8 cores), and shipped pre-transposed so every device-side matmul
contracts over the partition dimension. Each core returns
  yea/yeb: [D, capA/B] half-expert outputs, scaled by the combine weight (bf16)
  sh:      [T, D]      shared-expert partial (its H-shard, bf16)
Host scatters ye back by token index (the two halves of an expert sum via
the scatter-add) and sums the 8 sh partials — the output gather performs
the MoE combine; no on-device collectives needed.

Timing model (measured): the PE streams bf16 matmuls at ~0.507 ns/col
(P0 sustained clock ~1.97GHz, HAM warm) with LDWEIGHTS fully hidden, so
the kernel is a single gapless matmul stream; the wins over a naive
schedule are all in the first ~16us (framework preamble ~9us, DMA spin-up
to ~11.7us, HAM warmup 3.4us) and the last ~2us (output drain):
  - warmup matmuls start as early as possible: scratch is memset in two
    pieces (gpsimd does the 128-col slice the first matmuls need, vector
    does the rest) so the first warmup matmul has a minimal dependency.
  - xe ships token-chunk-major and w1 in per-h-block slabs of [128,KD,128]
    so the first real matmul needs only ~0.53MB landed, not 1.6MB.
  - all input streams trigger from the GpSimd queue; output (ye/sh)
    triggers stay on Sync. Input triggers then never convoy behind
    compute-gated output triggers (in-order queues).
  - the final shared-expert tile is split into two 256-col pieces so the
    last cast+DMA drain is ~0.7us shorter.

Device kernel (per core, TensorE-bound):
  warmup (flips the HAM clock gate during the initial DMAs; keep-alive
  matmuls gated on each early transfer stop the gate re-arming)
  B:  hT = gelu(W1half^T x_e)   for slot A then slot B; streamed h-slabs
  C:  ye = w * (W2half^T hT)    streamed d-slabs, tokens as the moving dim
  D:  hsT = gelu(Ws1^T x)       all T tokens through this core's H-shard
  E:  sh = Ws2^T hsT

Compute dtype: bf16 operands, fp32 PSUM accumulation (rel err ~3.6e-3).
fp8 was evaluated and rejected: e4m3's 3-bit mantissa gives ~5e-2 rel err
on these random-walk GEMM sums (no sqrt(K) averaging), over the 2e-2 gate.
"""

import hashlib
import sys

sys.path.insert(0, "/opt/trn_rl_repo")

import numpy as np
import ml_dtypes

import concourse.bass as bass
import concourse.bacc as bacc
import concourse.mybir as mybir
import concourse.tile as tile
from concourse.bass_utils import run_bass_kernel_spmd

BF16 = ml_dtypes.bfloat16
F32 = np.float32

T, D, E, TOP_K, H = 2048, 1024, 8, 2, 4096
H2 = H // 2          # half-expert hidden
HS = H // 8          # shared-expert hidden shard per core
KD = D // 128        # 8  k-chunks over D
KH2 = H2 // 128      # 16 k-chunks over a half-expert
KHS = HS // 128      # 4  k-chunks over the shared shard
N_CORES = 8

_DT = mybir.dt.bfloat16
_cache: dict = {}
_wcache: dict = {}


def _slotdims(n):
    """Equal token-chunks: (ntc, tsz) with ntc*tsz >= n, tsz <= 512, even."""
    n = max(64, n)
    ntc = -(-n // 512)
    tsz = -(-n // (2 * ntc)) * 2
    return ntc, tsz


def _build(dims):
    """Build + finalize the SPMD device program for ((ntcA,tszA),(ntcB,tszB))."""
    nc = bacc.Bacc("TRN2", target_bir_lowering=False, debug=False)

    caps = tuple(ntc * tsz for ntc, tsz in dims)
    xe_d, w1_d, w2_d, b1_d, wr_d, ye_d = {}, {}, {}, {}, {}, {}
    for s, (ntc, tsz), cap in zip("ab", dims, caps):
        xe_d[s] = nc.dram_tensor(f"xe{s}", [ntc, 128, KD, tsz], _DT, kind="ExternalInput")
        w1_d[s] = nc.dram_tensor(f"w1{s}", [KH2, 128, KD, 128], _DT, kind="ExternalInput")
        w2_d[s] = nc.dram_tensor(f"w2{s}", [8, 128, KH2, 128], _DT, kind="ExternalInput")
        b1_d[s] = nc.dram_tensor(f"b1{s}", [128, KH2], mybir.dt.float32, kind="ExternalInput")
        wr_d[s] = nc.dram_tensor(f"wr{s}", [128, cap], mybir.dt.float32, kind="ExternalInput")
        ye_d[s] = nc.dram_tensor(f"ye{s}", [D, cap], _DT, kind="ExternalOutput")
    xt_d = nc.dram_tensor("xt", [4, 128, KD, 512], _DT, kind="ExternalInput")
    ws1_d = nc.dram_tensor("ws1", [128, KD, HS], _DT, kind="ExternalInput")
    ws2_d = nc.dram_tensor("ws2", [128, KHS, D], _DT, kind="ExternalInput")
    bs1_d = nc.dram_tensor("bs1c", [128, KHS], mybir.dt.float32, kind="ExternalInput")
    sh_d = nc.dram_tensor("sh", [T, D], _DT, kind="ExternalOutput")

    gelu = mybir.ActivationFunctionType.Gelu

    with tile.TileContext(nc) as tc:
        with (
            tc.tile_pool(name="resident", bufs=1) as rpool,
            # bufs=4 keeps slab-DMA triggers hazard-free: with fewer bufs a
            # trigger carries a WAR hazard on an old slab's last matmul and
            # BLOCKS the in-order input queue, convoying later triggers
            tc.tile_pool(name="w1s", bufs=4) as w1pool,
            tc.tile_pool(name="w2s", bufs=4) as w2pool,
            tc.tile_pool(name="xts", bufs=3) as xtpool,
            tc.tile_pool(name="psum", bufs=7, space="PSUM") as pspool,
            tc.tile_pool(name="wpsum", bufs=1, space="PSUM") as wpspool,
            tc.tile_pool(name="outs", bufs=6) as opool,
        ):
            # ---- PE warmup: dummy matmuls while the first DMAs are in
            # flight, so the HAM clock gate is released for the real work.
            # The first two matmuls need only scratch[:,0:128], memset on
            # gpsimd (its queue is otherwise idle) so the PE starts with the
            # smallest possible dependency; vector fills the rest in parallel.
            scratch = rpool.tile([128, 512], _DT)
            nc.gpsimd.memset(scratch[:, 0:128], 0.0)
            nc.vector.memset(scratch[:, 128:512], 0.0)
            wps = wpspool.tile([128, 512], mybir.dt.float32)
            for _ in range(2):
                nc.tensor.matmul(wps[:, 0:128], scratch[:, 0:128], scratch[:, 0:128], start=True, stop=True)
            # The warmup must keep the PE continuously busy until the first
            # real matmul's data lands: any idle gap before the HAM
            # un-throttle resets the ~3.4us sustained-busy window and the
            # real work starts at 1.2GHz.
            for _ in range(9):
                nc.tensor.matmul(wps[:], scratch[:, 0:128], scratch[:], start=True, stop=True)
            for _ in range(3):
                nc.tensor.matmul(wps[:, 0:128], scratch[:, 0:128], scratch[:, 0:128], start=True, stop=True)

            # ---- phase-B-critical loads, in first-consumption order, all
            # on the gpsimd queue (inputs) so they never convoy behind
            # compute-gated output triggers on Sync.
            (ntca, tsza), (ntcb, tszb) = dims
            xe_sb = {"a": [], "b": []}
            wr_sb, b1_sb, hT = {}, {}, {}
            for i in range(ntca):
                xe_sb["a"].append(rpool.tile([128, KD, tsza], _DT, name=f"xea{i}"))
            nc.gpsimd.dma_start(xe_sb["a"][0][:], xe_d["a"][0])
            w1s0 = w1pool.tile([128, KD, 128], _DT)
            nc.gpsimd.dma_start(w1s0[:], w1_d["a"][0])
            for i in range(1, ntca):
                nc.gpsimd.dma_start(xe_sb["a"][i][:], xe_d["a"][i])
            b1_sb["a"] = rpool.tile([128, KH2], mybir.dt.float32, name="b1a", tag="b1a")
            nc.gpsimd.dma_start(b1_sb["a"][:], b1_d["a"][:])

            # keep-alive touches: one tiny matmul gated on each early
            # transfer, so no single late transfer leaves the PE idle long
            # enough to re-arm the HAM clock gate
            nc.tensor.matmul(wps[:, 0:128], xe_sb["a"][0][:, 0, 0:128],
                             scratch[:, 0:128], start=True, stop=True)
            nc.tensor.matmul(wps[:, 0:128], w1s0[:, 0, :],
                             scratch[:, 0:128], start=True, stop=True)
            if ntca > 1:
                nc.tensor.matmul(wps[:, 0:128], xe_sb["a"][1][:, 0, 0:128],
                                 scratch[:, 0:128], start=True, stop=True)

            hT["a"] = rpool.tile([128, KH2, caps[0]], _DT, name="hTa", tag="hTa")
            hT["b"] = rpool.tile([128, KH2, caps[1]], _DT, name="hTb", tag="hTb")
            hsT = rpool.tile([128, KHS, T], _DT)

            # ---- phase B: routed GEMM1 per slot ----
            for si, s in enumerate("ab"):
                ntc, tsz = dims[si]
                if s == "b":
                    for i in range(ntcb):
                        xe_sb["b"].append(rpool.tile([128, KD, tszb], _DT, name=f"xeb{i}"))
                        nc.gpsimd.dma_start(xe_sb["b"][i][:], xe_d["b"][i])
                    b1_sb["b"] = rpool.tile([128, KH2], mybir.dt.float32, name="b1b", tag="b1b")
                    nc.gpsimd.dma_start(b1_sb["b"][:], b1_d["b"][:])
                for h in range(KH2):
                    if s == "a" and h == 0:
                        w1s = w1s0
                    else:
                        w1s = w1pool.tile([128, KD, 128], _DT)
                        nc.gpsimd.dma_start(w1s[:], w1_d[s][h])
                    for t in range(ntc):
                        ps = pspool.tile([128, 512], mybir.dt.float32)
                        for k in range(KD):
                            nc.tensor.matmul(
                                ps[:, :tsz],
                                w1s[:, k, :],
                                xe_sb[s][t][:, k, :],
                                start=(k == 0),
                                stop=(k == KD - 1),
                            )
                        nc.scalar.activation(
                            hT[s][:, h, t * tsz:(t + 1) * tsz], ps[:, :tsz], gelu,
                            bias=b1_sb[s][:, h:h + 1],
                        )

            # ---- phase C: routed GEMM2 (tokens moving) + weight scale ----
            ws1_sb = rpool.tile([128, KD, HS], _DT)
            ws2_sb = rpool.tile([128, KHS, D], _DT)
            bs1_sb = rpool.tile([128, KHS], mybir.dt.float32)
            for si, s in enumerate("ab"):
                ntc, tsz = dims[si]
                wr_sb[s] = rpool.tile([128, caps[si]], mybir.dt.float32, name=f"wr{s}", tag=f"wr{s}")
                nc.gpsimd.dma_start(wr_sb[s][:], wr_d[s][:])
                for d in range(8):
                    w2s = w2pool.tile([128, KH2, 128], _DT)
                    nc.gpsimd.dma_start(w2s[:], w2_d[s][d])
                    # shared-expert loads trickle behind the early slabs
                    if s == "a" and d == 0:
                        nc.gpsimd.dma_start(ws1_sb[:], ws1_d[:])
                    elif s == "a" and d == 1:
                        nc.gpsimd.dma_start(ws2_sb[:], ws2_d[:])
                    elif s == "a" and d == 2:
                        nc.gpsimd.dma_start(bs1_sb[:], bs1_d[:])
                    for t in range(ntc):
                        ps = pspool.tile([128, 512], mybir.dt.float32)
                        for k in range(KH2):
                            nc.tensor.matmul(
                                ps[:, :tsz],
                                w2s[:, k, :],
                                hT[s][:, k, t * tsz:(t + 1) * tsz],
                                start=(k == 0),
                                stop=(k == KH2 - 1),
                            )
                        eo = opool.tile([128, 512], _DT, tag="eo")
                        nc.vector.tensor_mul(
                            eo[:, :tsz], ps[:, :tsz], wr_sb[s][:, t * tsz:(t + 1) * tsz]
                        )
                        nc.sync.dma_start(
                            ye_d[s][d * 128:(d + 1) * 128, t * tsz:(t + 1) * tsz],
                            eo[:, :tsz],
                        )

            # ---- phase D: shared GEMM1 over all T tokens ----
            for tcn in range(4):
                xts = xtpool.tile([128, KD, 512], _DT)
                nc.gpsimd.dma_start(xts[:], xt_d[tcn])
                for hs in range(KHS):
                    ps = pspool.tile([128, 512], mybir.dt.float32)
                    for k in range(KD):
                        nc.tensor.matmul(
                            ps[:],
                            ws1_sb[:, k, hs * 128:(hs + 1) * 128],
                            xts[:, k, :],
                            start=(k == 0),
                            stop=(k == KD - 1),
                        )
                    nc.scalar.activation(
                        hsT[:, hs, tcn * 512:(tcn + 1) * 512], ps[:], gelu,
                        bias=bs1_sb[:, hs:hs + 1],
                    )

            # ---- phase E: shared GEMM2 ----
            for t in range(T // 128):
                for dh in range(2):
                    if t == T // 128 - 1 and dh == 1:
                        # final tile in two 256-col pieces: the tail drain
                        # after the last matmul is one 256-col cast + one
                        # trigger instead of a full 512-col chain
                        for c in range(2):
                            ps = pspool.tile([128, 512], mybir.dt.float32)
                            c0 = dh * 512 + c * 256
                            for k in range(KHS):
                                nc.tensor.matmul(
                                    ps[:, 0:256],
                                    hsT[:, k, t * 128:(t + 1) * 128],
                                    ws2_sb[:, k, c0:c0 + 256],
                                    start=(k == 0),
                                    stop=(k == KHS - 1),
                                )
                            so = opool.tile([128, 512], _DT, tag="so")
                            nc.vector.tensor_copy(so[:, 0:256], ps[:, 0:256])
                            nc.sync.dma_start(
                                sh_d[t * 128:(t + 1) * 128, c0:c0 + 256],
                                so[:, 0:256],
                            )
                        continue
                    ps = pspool.tile([128, 512], mybir.dt.float32)
                    for k in range(KHS):
                        nc.tensor.matmul(
                            ps[:],
                            hsT[:, k, t * 128:(t + 1) * 128],
                            ws2_sb[:, k, dh * 512:(dh + 1) * 512],
                            start=(k == 0),
                            stop=(k == KHS - 1),
                        )
                    so = opool.tile([128, 512], _DT, tag="so")
                    # plain vector cast: the scalar engine sleeps after
                    # phase D and wakes ~0.5us late, gating the closing DMA
                    nc.vector.tensor_copy(so[:], ps[:])
                    nc.sync.dma_start(
                        sh_d[t * 128:(t + 1) * 128, dh * 512:(dh + 1) * 512], so[:]
                    )

    nc.finalize()
    return nc


def _routing(xf, Wg, bg, bias):
    """Host gate: fp64 for a stable top-2 ranking (matches fp32 reference
    ordering except for ~1e-7-wide ties, which don't occur at these margins)."""
    logits = xf.astype(np.float64) @ Wg.T.astype(np.float64) + bg + bias
    scores = (1.0 / (1.0 + np.exp(-logits))).astype(np.float32)
    # stable sort => ties break toward the lower expert index, like lax.top_k
    top_idx = np.argsort(-scores, axis=1, kind="stable")[:, :TOP_K]
    top_w = np.take_along_axis(scores, top_idx, axis=1)
    return top_idx, top_w


def kernel(x, Wg, bg, bias, W1, b1, W2, b2, Ws1, bs1, Ws2, bs2):
    x = np.asarray(x, F32)
    Wg, bg, bias = np.asarray(Wg, F32), np.asarray(bg, F32), np.asarray(bias, F32)
    W1, b1 = np.asarray(W1, F32), np.asarray(b1, F32)
    W2, b2 = np.asarray(W2, F32), np.asarray(b2, F32)
    Ws1, bs1 = np.asarray(Ws1, F32), np.asarray(bs1, F32)
    Ws2, bs2 = np.asarray(Ws2, F32), np.asarray(bs2, F32)

    xf = x.reshape(-1, D)
    top_idx, top_w = _routing(xf, Wg, bg, bias)

    sels, ws = [], []
    for e in range(E):
        pick = (top_idx == e)
        sel = np.where(pick.any(axis=1))[0]
        w = np.where(pick[sel, 0], top_w[sel, 0], top_w[sel, 1]).astype(F32)
        sels.append(sel)
        ws.append(w)
    counts = np.array([len(s) for s in sels])
    order = np.argsort(-counts, kind="stable")
    heavy, light = order[:4], order[4:]
    dims = (_slotdims(counts[heavy].max()), _slotdims(counts[light].max()))
    caps = tuple(ntc * tsz for ntc, tsz in dims)

    if dims not in _cache:
        _cache[dims] = _build(dims)
    nc = _cache[dims]

    x_bf = xf.astype(BF16)
    # xt: [4, 128, KD, 512]  (token-chunk major, partition-major inside)
    xt = np.ascontiguousarray(
        x_bf.T.reshape(KD, 128, 4, 512).transpose(2, 1, 0, 3)
    )

    # Half-expert weight re-layouts are input-independent; cache across calls
    # (keyed by content hash, so a reused buffer can't serve stale layouts).
    hsh = hashlib.blake2b(digest_size=16)
    for a in (W1, W2, Ws1, Ws2, b1, bs1):
        hsh.update(np.ascontiguousarray(a).data)
    wkey = hsh.hexdigest()
    wmaps = _wcache.get(wkey)
    if wmaps is None:
        wmaps = {"half": {}, "core": []}
        for e in range(E):
            for hf in range(2):
                r0 = hf * H2
                wmaps["half"][(e, hf)] = {
                    # W1 half rows -> W1halfT [D, H2] -> [KH2, 128, KD, 128]
                    "w1": np.ascontiguousarray(
                        W1[e][r0:r0 + H2].T.reshape(KD, 128, KH2, 128)
                        .transpose(2, 1, 0, 3).astype(BF16)
                    ),
                    # W2 half cols -> W2halfT [H2, D] -> [8, 128, KH2, 128]
                    "w2": np.ascontiguousarray(
                        W2[e][:, r0:r0 + H2].T.reshape(KH2, 128, 8, 128)
                        .transpose(2, 1, 0, 3).astype(BF16)
                    ),
                    "b1": np.ascontiguousarray(b1[e][r0:r0 + H2].reshape(KH2, 128).T),
                }
        for c in range(N_CORES):
            hs0 = c * HS
            wmaps["core"].append({
                "ws1": np.ascontiguousarray(
                    Ws1[hs0:hs0 + HS].T.reshape(KD, 128, HS)
                    .transpose(1, 0, 2).astype(BF16)
                ),
                "ws2": np.ascontiguousarray(
                    Ws2[:, hs0:hs0 + HS].T.reshape(KHS, 128, D)
                    .transpose(1, 0, 2).astype(BF16)
                ),
                "bs1c": np.ascontiguousarray(bs1[hs0:hs0 + HS].reshape(KHS, 128).T),
            })
        _wcache.clear()
        _wcache[wkey] = wmaps

    # per-expert gathered tokens + combine weights at the slot cap
    def gathered(e, ntc, tsz):
        cap = ntc * tsz
        sel, w = sels[e], ws[e]
        xe = np.zeros((cap, D), BF16)
        xe[: len(sel)] = x_bf[sel]
        xe_t = np.ascontiguousarray(
            xe.T.reshape(KD, 128, ntc, tsz).transpose(2, 1, 0, 3)
        )
        wpad = np.zeros(cap, F32)
        wpad[: len(w)] = w
        wr = np.ascontiguousarray(np.broadcast_to(wpad, (128, cap)))
        return xe_t, wr

    gcache = {}
    in_maps = []
    for c in range(N_CORES):
        m = {"xt": xt, **wmaps["core"][c]}
        for s, grp, (ntc, tsz) in (("a", heavy, dims[0]), ("b", light, dims[1])):
            e, hf = int(grp[c // 2]), c % 2
            if e not in gcache:
                gcache[e] = gathered(e, ntc, tsz)
            m[f"xe{s}"], m[f"wr{s}"] = gcache[e]
            half = wmaps["half"][(e, hf)]
            m[f"w1{s}"], m[f"w2{s}"], m[f"b1{s}"] = half["w1"], half["w2"], half["b1"]
        in_maps.append(m)

    res = run_bass_kernel_spmd(nc, in_maps, core_ids=list(range(N_CORES)))

    out = np.zeros((T, D), F32)
    for c in range(N_CORES):
        out += res.results[c]["sh"].astype(F32)
        for s, grp in (("a", heavy), ("b", light)):
            e = int(grp[c // 2])
            sel = sels[e]
            out[sel] += res.results[c][f"ye{s}"][:, : len(sel)].T.astype(F32)
    # biases handled host-side: per-token weighted b2, plus bs2
    wdense = np.zeros((T, E), F32)
    np.put_along_axis(wdense, top_idx, top_w, axis=1)
    out += wdense @ b2
    out += bs2
    return out.reshape(x.shape)


# revision 12
# speedup vs baseline: 1.2199x; 1.2199x over previous
"""DeepSeekMoE kernel for 8 TRN2 NeuronCores.

Sharding: load-balanced expert-parallel. Each routed expert's FFN is split
in half along the hidden (H) axis across two cores, and the 4 heaviest
experts (by routed-token count) are paired with the 4 lightest, so every
core carries one heavy half-expert (slot A) and one light half-expert
(slot B) — per-core matmul rows become nearly uniform instead of every
core paying the max expert's padding. Each core also owns a 1/8 H-shard
of the shared expert (tensor-parallel).

The tiny gate (sigmoid + top-2 over E=8) runs on host; tokens are gathered
per expert, padded to ntc*tsz (equal token-chunks <=512, SPMD: one program
for all 8 cores), and shipped pre-transposed so every device-side matmul
contracts over the partition dimension. Each core returns
  yea/yeb: [D, capA/B] half-expert outputs, scaled by the combine weight (bf16)
  sh:      [T, D]      shared-expert partial (its H-shard, bf16)
Host scatters ye back by token index (the two halves of an expert sum via
the scatter-add) and sums the 8 sh partials — the output gather performs
the MoE combine; no on-device collectives needed.

Timing model (measured): the PE streams bf16 matmuls at ~0.507 ns/col
(P0 sustained clock ~1.97GHz, HAM warm) with LDWEIGHTS fully hidden, so
the kernel is a single gapless matmul stream; the wins over a naive
schedule are all in the first ~16us (framework preamble ~9us, DMA spin-up
to ~11.7us, HAM warmup 3.4us) and the last ~2us (output drain):
  - warmup matmuls start as early as possible: scratch is memset in two
    pieces (gpsimd does the 128-col slice the first matmuls need, vector
    does the rest) so the first warmup matmul has a minimal dependency.
  - xe ships token-chunk-major and w1 in per-h-block slabs of [128,KD,128]
    so the first real matmul needs only ~0.53MB landed, not 1.6MB.
  - all input streams trigger from the GpSimd queue; output (ye/sh)
    triggers stay on Sync. Input triggers then never convoy behind
    compute-gated output triggers (in-order queues).
  - the final shared-expert tile is split into two 256-col pieces so the
    last cast+DMA drain is ~0.7us shorter.

Device kernel (per core, TensorE-bound):
  warmup (flips the HAM clock gate during the initial DMAs; keep-alive
  matmuls gated on each early transfer stop the gate re-arming)
  B:  hT = gelu(W1half^T x_e)   for slot A then slot B; streamed h-slabs
  C:  ye = w * (W2half^T hT)    streamed d-slabs, tokens as the moving dim
  D:  hsT = gelu(Ws1^T x)       all T tokens through this core's H-shard
  E:  sh = Ws2^T hsT

Compute dtype: bf16 operands, fp32 PSUM accumulation (rel err ~3.6e-3).
fp8 was evaluated and rejected: e4m3's 3-bit mantissa gives ~5e-2 rel err
on these random-walk GEMM sums (no sqrt(K) averaging), over the 2e-2 gate.
"""

import hashlib
import sys

sys.path.insert(0, "/opt/trn_rl_repo")

import numpy as np
import ml_dtypes

import concourse.bass as bass
import concourse.bacc as bacc
import concourse.mybir as mybir
import concourse.tile as tile
from concourse.bass_utils import run_bass_kernel_spmd

BF16 = ml_dtypes.bfloat16
F32 = np.float32

T, D, E, TOP_K, H = 2048, 1024, 8, 2, 4096
H2 = H // 2          # half-expert hidden
HS = H // 8          # shared-expert hidden shard per core
KD = D // 128        # 8  k-chunks over D
KH2 = H2 // 128      # 16 k-chunks over a half-expert
KHS = HS // 128      # 4  k-chunks over the shared shard
N_CORES = 8

_DT = mybir.dt.bfloat16
_cache: dict = {}
_wcache: dict = {}


def _slotdims(n):
    """Equal token-chunks: (ntc, tsz) with ntc*tsz >= n, tsz <= 512.
    tsz is a multiple of 8 so bf16 moving-operand rows stay 16B-aligned —
    unaligned rows (e.g. tsz=270) measurably slow the PE stream by ~3%."""
    n = max(64, n)
    ntc = -(-n // 512)
    tsz = -(-n // (8 * ntc)) * 8
    return ntc, tsz


def _build(dims):
    """Build + finalize the SPMD device program for ((ntcA,tszA),(ntcB,tszB))."""
    nc = bacc.Bacc("TRN2", target_bir_lowering=False, debug=False)

    caps = tuple(ntc * tsz for ntc, tsz in dims)
    xe_d, w1_d, w2_d, b1_d, wr_d, ye_d = {}, {}, {}, {}, {}, {}
    w1f_d = {}
    for s, (ntc, tsz), cap in zip("ab", dims, caps):
        xe_d[s] = nc.dram_tensor(f"xe{s}", [ntc, 128, KD, tsz], _DT, kind="ExternalInput")
        # h-block 0 duplicated as its own small tensor: the first real matmul
        # then needs only 0.25MB of weights + one xe chunk, landing ~1.5us
        # earlier than a full 0.5MB slab would
        w1f_d[s] = nc.dram_tensor(f"w1f{s}", [128, KD, 128], _DT, kind="ExternalInput")
        w1_d[s] = nc.dram_tensor(f"w1{s}", [KH2 // 2, 128, KD, 256], _DT, kind="ExternalInput")
        w2_d[s] = nc.dram_tensor(f"w2{s}", [8, 128, KH2, 128], _DT, kind="ExternalInput")
        b1_d[s] = nc.dram_tensor(f"b1{s}", [128, KH2], mybir.dt.float32, kind="ExternalInput")
        wr_d[s] = nc.dram_tensor(f"wr{s}", [128, cap], mybir.dt.float32, kind="ExternalInput")
        ye_d[s] = nc.dram_tensor(f"ye{s}", [D, cap], _DT, kind="ExternalOutput")
    xt_d = nc.dram_tensor("xt", [4, 128, KD, 512], _DT, kind="ExternalInput")
    ws1_d = nc.dram_tensor("ws1", [128, KD, HS], _DT, kind="ExternalInput")
    ws2_d = nc.dram_tensor("ws2", [128, KHS, D], _DT, kind="ExternalInput")
    bs1_d = nc.dram_tensor("bs1c", [128, KHS], mybir.dt.float32, kind="ExternalInput")
    sh_d = nc.dram_tensor("sh", [T, D], _DT, kind="ExternalOutput")

    gelu = mybir.ActivationFunctionType.Gelu

    with tile.TileContext(nc) as tc:
        with (
            tc.tile_pool(name="resident", bufs=1) as rpool,
            # bufs=4 keeps slab-DMA triggers hazard-free: with fewer bufs a
            # trigger carries a WAR hazard on an old slab's last matmul and
            # BLOCKS the in-order input queue, convoying later triggers
            tc.tile_pool(name="w1s", bufs=4) as w1pool,
            tc.tile_pool(name="w2s", bufs=4) as w2pool,
            tc.tile_pool(name="xts", bufs=3) as xtpool,
            tc.tile_pool(name="psum", bufs=7, space="PSUM") as pspool,
            tc.tile_pool(name="wpsum", bufs=1, space="PSUM") as wpspool,
            tc.tile_pool(name="outs", bufs=6) as opool,
        ):
            # ---- PE warmup: dummy matmuls while the first DMAs are in
            # flight, so the HAM clock gate is released for the real work.
            # The first two matmuls need only scratch[:,0:128], memset on
            # gpsimd (its queue is otherwise idle) so the PE starts with the
            # smallest possible dependency; vector fills the rest in parallel.
            scratch = rpool.tile([128, 512], _DT)
            nc.gpsimd.memset(scratch[:, 0:128], 0.0)
            nc.vector.memset(scratch[:, 128:512], 0.0)
            wps = wpspool.tile([128, 512], mybir.dt.float32)
            for _ in range(2):
                nc.tensor.matmul(wps[:, 0:128], scratch[:, 0:128], scratch[:, 0:128], start=True, stop=True)
            # The warmup must keep the PE continuously busy until the first
            # real matmul's data lands: any idle gap before the HAM
            # un-throttle resets the ~3.4us sustained-busy window and the
            # real work starts at 1.2GHz.
            for _ in range(9):
                nc.tensor.matmul(wps[:], scratch[:, 0:128], scratch[:], start=True, stop=True)
            for _ in range(3):
                nc.tensor.matmul(wps[:, 0:128], scratch[:, 0:128], scratch[:, 0:128], start=True, stop=True)

            # ---- phase-B-critical loads, in first-consumption order, all
            # on the gpsimd queue (inputs) so they never convoy behind
            # compute-gated output triggers on Sync.
            (ntca, tsza), (ntcb, tszb) = dims
            xe_sb = {"a": [], "b": []}
            wr_sb, b1_sb, hT = {}, {}, {}
            for i in range(ntca):
                xe_sb["a"].append(rpool.tile([128, KD, tsza], _DT, name=f"xea{i}"))
            nc.gpsimd.dma_start(xe_sb["a"][0][:], xe_d["a"][0])
            w1f = {"a": rpool.tile([128, KD, 128], _DT, name="w1fa")}
            nc.gpsimd.dma_start(w1f["a"][:], w1f_d["a"][:])
            for i in range(1, ntca):
                nc.gpsimd.dma_start(xe_sb["a"][i][:], xe_d["a"][i])
            b1_sb["a"] = rpool.tile([128, KH2], mybir.dt.float32, name="b1a", tag="b1a")
            nc.gpsimd.dma_start(b1_sb["a"][:], b1_d["a"][:])

            # keep-alive touches: one tiny matmul gated on each early
            # transfer, so no single late transfer leaves the PE idle long
            # enough to re-arm the HAM clock gate
            nc.tensor.matmul(wps[:, 0:128], xe_sb["a"][0][:, 0, 0:128],
                             scratch[:, 0:128], start=True, stop=True)
            nc.tensor.matmul(wps[:, 0:128], w1f["a"][:, 0, :],
                             scratch[:, 0:128], start=True, stop=True)
            if ntca > 1:
                nc.tensor.matmul(wps[:, 0:128], xe_sb["a"][1][:, 0, 0:128],
                                 scratch[:, 0:128], start=True, stop=True)

            hT["a"] = rpool.tile([128, KH2, caps[0]], _DT, name="hTa", tag="hTa")
            hT["b"] = rpool.tile([128, KH2, caps[1]], _DT, name="hTb", tag="hTb")
            hsT = rpool.tile([128, KHS, T], _DT)

            # ---- phase B: routed GEMM1 per slot ----
            for si, s in enumerate("ab"):
                ntc, tsz = dims[si]
                if s == "b":
                    for i in range(ntcb):
                        xe_sb["b"].append(rpool.tile([128, KD, tszb], _DT, name=f"xeb{i}"))
                        nc.gpsimd.dma_start(xe_sb["b"][i][:], xe_d["b"][i])
                    b1_sb["b"] = rpool.tile([128, KH2], mybir.dt.float32, name="b1b", tag="b1b")
                    nc.gpsimd.dma_start(b1_sb["b"][:], b1_d["b"][:])
                    w1f["b"] = rpool.tile([128, KD, 128], _DT, name="w1fb")
                    nc.gpsimd.dma_start(w1f["b"][:], w1f_d["b"][:])
                slab = None
                for h in range(KH2):
                    # h=0 comes from the small fast-path tensor; h>=1 from
                    # 0.5MB slabs (big slabs keep the input queue's WAR
                    # lookahead ~4.4us so later streams never convoy)
                    if h == 1 or (h >= 2 and h % 2 == 0):
                        slab = w1pool.tile([128, KD, 256], _DT)
                        nc.gpsimd.dma_start(slab[:], w1_d[s][h // 2])
                    hh = h % 2
                    for t in range(ntc):
                        ps = pspool.tile([128, 512], mybir.dt.float32)
                        for k in range(KD):
                            nc.tensor.matmul(
                                ps[:, :tsz],
                                w1f[s][:, k, :] if h == 0
                                else slab[:, k, hh * 128:hh * 128 + 128],
                                xe_sb[s][t][:, k, :],
                                start=(k == 0),
                                stop=(k == KD - 1),
                            )
                        nc.scalar.activation(
                            hT[s][:, h, t * tsz:(t + 1) * tsz], ps[:, :tsz], gelu,
                            bias=b1_sb[s][:, h:h + 1],
                        )

            # ---- phase C: routed GEMM2 (tokens moving) + weight scale ----
            ws1_sb = rpool.tile([128, KD, HS], _DT)
            ws2_sb = rpool.tile([128, KHS, D], _DT)
            bs1_sb = rpool.tile([128, KHS], mybir.dt.float32)
            for si, s in enumerate("ab"):
                ntc, tsz = dims[si]
                wr_sb[s] = rpool.tile([128, caps[si]], mybir.dt.float32, name=f"wr{s}", tag=f"wr{s}")
                nc.gpsimd.dma_start(wr_sb[s][:], wr_d[s][:])
                for d in range(8):
                    w2s = w2pool.tile([128, KH2, 128], _DT)
                    nc.gpsimd.dma_start(w2s[:], w2_d[s][d])
                    # shared-expert loads trickle behind the early slabs
                    if s == "a" and d == 0:
                        nc.gpsimd.dma_start(ws1_sb[:], ws1_d[:])
                    elif s == "a" and d == 1:
                        nc.gpsimd.dma_start(ws2_sb[:], ws2_d[:])
                    elif s == "a" and d == 2:
                        nc.gpsimd.dma_start(bs1_sb[:], bs1_d[:])
                    for t in range(ntc):
                        ps = pspool.tile([128, 512], mybir.dt.float32)
                        for k in range(KH2):
                            nc.tensor.matmul(
                                ps[:, :tsz],
                                w2s[:, k, :],
                                hT[s][:, k, t * tsz:(t + 1) * tsz],
                                start=(k == 0),
                                stop=(k == KH2 - 1),
                            )
                        eo = opool.tile([128, 512], _DT, tag="eo")
                        nc.vector.tensor_mul(
                            eo[:, :tsz], ps[:, :tsz], wr_sb[s][:, t * tsz:(t + 1) * tsz]
                        )
                        nc.sync.dma_start(
                            ye_d[s][d * 128:(d + 1) * 128, t * tsz:(t + 1) * tsz],
                            eo[:, :tsz],
                        )

            # ---- phase D: shared GEMM1 over all T tokens ----
            for tcn in range(4):
                xts = xtpool.tile([128, KD, 512], _DT)
                nc.gpsimd.dma_start(xts[:], xt_d[tcn])
                for hs in range(KHS):
                    ps = pspool.tile([128, 512], mybir.dt.float32)
                    for k in range(KD):
                        nc.tensor.matmul(
                            ps[:],
                            ws1_sb[:, k, hs * 128:(hs + 1) * 128],
                            xts[:, k, :],
                            start=(k == 0),
                            stop=(k == KD - 1),
                        )
                    nc.scalar.activation(
                        hsT[:, hs, tcn * 512:(tcn + 1) * 512], ps[:], gelu,
                        bias=bs1_sb[:, hs:hs + 1],
                    )

            # ---- phase E: shared GEMM2 ----
            for t in range(T // 128):
                for dh in range(2):
                    ps = pspool.tile([128, 512], mybir.dt.float32)
                    for k in range(KHS):
                        nc.tensor.matmul(
                            ps[:],
                            hsT[:, k, t * 128:(t + 1) * 128],
                            ws2_sb[:, k, dh * 512:(dh + 1) * 512],
                            start=(k == 0),
                            stop=(k == KHS - 1),
                        )
                    so = opool.tile([128, 512], _DT, tag="so")
                    # the last four tiles alternate scalar/vector casts with
                    # triggers on scalar/Sync respectively: the closing
                    # cast+trigger chains run on two queue pairs in parallel
                    # (scalar wakes at t=14, hiding its ~0.5us wake latency
                    # before the drain-critical final tiles)
                    if t >= T // 128 - 2 and dh == 0:
                        nc.scalar.copy(so[:], ps[:])
                        nc.scalar.dma_start(
                            sh_d[t * 128:(t + 1) * 128, dh * 512:(dh + 1) * 512],
                            so[:],
                        )
                    else:
                        nc.vector.tensor_copy(so[:], ps[:])
                        nc.sync.dma_start(
                            sh_d[t * 128:(t + 1) * 128, dh * 512:(dh + 1) * 512],
                            so[:],
                        )

    nc.finalize()
    return nc


def _routing(xf, Wg, bg, bias):
    """Host gate: fp64 for a stable top-2 ranking (matches fp32 reference
    ordering except for ~1e-7-wide ties, which don't occur at these margins)."""
    logits = xf.astype(np.float64) @ Wg.T.astype(np.float64) + bg + bias
    scores = (1.0 / (1.0 + np.exp(-logits))).astype(np.float32)
    # stable sort => ties break toward the lower expert index, like lax.top_k
    top_idx = np.argsort(-scores, axis=1, kind="stable")[:, :TOP_K]
    top_w = np.take_along_axis(scores, top_idx, axis=1)
    return top_idx, top_w


def kernel(x, Wg, bg, bias, W1, b1, W2, b2, Ws1, bs1, Ws2, bs2):
    x = np.asarray(x, F32)
    Wg, bg, bias = np.asarray(Wg, F32), np.asarray(bg, F32), np.asarray(bias, F32)
    W1, b1 = np.asarray(W1, F32), np.asarray(b1, F32)
    W2, b2 = np.asarray(W2, F32), np.asarray(b2, F32)
    Ws1, bs1 = np.asarray(Ws1, F32), np.asarray(bs1, F32)
    Ws2, bs2 = np.asarray(Ws2, F32), np.asarray(bs2, F32)

    xf = x.reshape(-1, D)
    top_idx, top_w = _routing(xf, Wg, bg, bias)

    sels, ws = [], []
    for e in range(E):
        pick = (top_idx == e)
        sel = np.where(pick.any(axis=1))[0]
        w = np.where(pick[sel, 0], top_w[sel, 0], top_w[sel, 1]).astype(F32)
        sels.append(sel)
        ws.append(w)
    counts = np.array([len(s) for s in sels])
    order = np.argsort(-counts, kind="stable")
    heavy, light = order[:4], order[4:]
    dims = (_slotdims(counts[heavy].max()), _slotdims(counts[light].max()))
    caps = tuple(ntc * tsz for ntc, tsz in dims)

    if dims not in _cache:
        _cache[dims] = _build(dims)
    nc = _cache[dims]

    x_bf = xf.astype(BF16)
    # xt: [4, 128, KD, 512]  (token-chunk major, partition-major inside)
    xt = np.ascontiguousarray(
        x_bf.T.reshape(KD, 128, 4, 512).transpose(2, 1, 0, 3)
    )

    # Half-expert weight re-layouts are input-independent; cache across calls
    # (keyed by content hash, so a reused buffer can't serve stale layouts).
    hsh = hashlib.blake2b(digest_size=16)
    for a in (W1, W2, Ws1, Ws2, b1, bs1):
        hsh.update(np.ascontiguousarray(a).data)
    wkey = hsh.hexdigest()
    wmaps = _wcache.get(wkey)
    if wmaps is None:
        wmaps = {"half": {}, "core": []}
        for e in range(E):
            for hf in range(2):
                r0 = hf * H2
                w1t = (
                    W1[e][r0:r0 + H2].T.reshape(KD, 128, KH2 // 2, 256)
                    .transpose(2, 1, 0, 3).astype(BF16)
                )
                wmaps["half"][(e, hf)] = {
                    # W1 half rows -> W1halfT [D, H2] -> [8, 128, KD, 256]
                    "w1": np.ascontiguousarray(w1t),
                    # h-block 0 duplicated for the fast first-tile path
                    "w1f": np.ascontiguousarray(w1t[0][:, :, 0:128]),
                    # W2 half cols -> W2halfT [H2, D] -> [8, 128, KH2, 128]
                    "w2": np.ascontiguousarray(
                        W2[e][:, r0:r0 + H2].T.reshape(KH2, 128, 8, 128)
                        .transpose(2, 1, 0, 3).astype(BF16)
                    ),
                    "b1": np.ascontiguousarray(b1[e][r0:r0 + H2].reshape(KH2, 128).T),
                }
        for c in range(N_CORES):
            hs0 = c * HS
            wmaps["core"].append({
                "ws1": np.ascontiguousarray(
                    Ws1[hs0:hs0 + HS].T.reshape(KD, 128, HS)
                    .transpose(1, 0, 2).astype(BF16)
                ),
                "ws2": np.ascontiguousarray(
                    Ws2[:, hs0:hs0 + HS].T.reshape(KHS, 128, D)
                    .transpose(1, 0, 2).astype(BF16)
                ),
                "bs1c": np.ascontiguousarray(bs1[hs0:hs0 + HS].reshape(KHS, 128).T),
            })
        _wcache.clear()
        _wcache[wkey] = wmaps

    # per-expert gathered tokens + combine weights at the slot cap
    def gathered(e, ntc, tsz):
        cap = ntc * tsz
        sel, w = sels[e], ws[e]
        xe = np.zeros((cap, D), BF16)
        xe[: len(sel)] = x_bf[sel]
        xe_t = np.ascontiguousarray(
            xe.T.reshape(KD, 128, ntc, tsz).transpose(2, 1, 0, 3)
        )
        wpad = np.zeros(cap, F32)
        wpad[: len(w)] = w
        wr = np.ascontiguousarray(np.broadcast_to(wpad, (128, cap)))
        return xe_t, wr

    gcache = {}
    in_maps = []
    for c in range(N_CORES):
        m = {"xt": xt, **wmaps["core"][c]}
        for s, grp, (ntc, tsz) in (("a", heavy, dims[0]), ("b", light, dims[1])):
            e, hf = int(grp[c // 2]), c % 2
            if e not in gcache:
                gcache[e] = gathered(e, ntc, tsz)
            m[f"xe{s}"], m[f"wr{s}"] = gcache[e]
            half = wmaps["half"][(e, hf)]
            m[f"w1{s}"], m[f"w2{s}"], m[f"b1{s}"] = half["w1"], half["w2"], half["b1"]
            m[f"w1f{s}"] = half["w1f"]
        in_maps.append(m)

    res = run_bass_kernel_spmd(nc, in_maps, core_ids=list(range(N_CORES)))

    out = np.zeros((T, D), F32)
    for c in range(N_CORES):
        out += res.results[c]["sh"].astype(F32)
        for s, grp in (("a", heavy), ("b", light)):
            e = int(grp[c // 2])
            sel = sels[e]
            out[sel] += res.results[c][f"ye{s}"][:, : len(sel)].T.astype(F32)
    # biases handled host-side: per-token weighted b2, plus bs2
    wdense = np.zeros((T, E), F32)
    np.put_along_axis(wdense, top_idx, top_w, axis=1)
    out += wdense @ b2
    out += bs2
    return out.reshape(x.shape)
